# revision 60
# baseline (speedup 1.0000x reference)
"""DCNv2 deformable conv kernel for TRN2 (Bass/Tile), one image per core.

v3: cost-model-driven restructure of the v2 pipeline.
- base grid + floor-trick bias folded into per-partition Act biases on the
  offset-conv PSUM (no baseC table, no omc add, no clip chain; offsets are
  conv outputs ~N(0, 0.03) so sample positions stay inside the padded
  table by >10 sigma).
- compact om layout: y rows 0-17 (9g+t), x rows 18-35, mask rows 36-53.
  idx = 60*y0 + (x0+122) via one fused scalar_tensor_tensor.
- gathers: 3-tap merged ap_gather per (half, row-class): out [128, 4704]
  u32 amortizes the table-scan term of the Pool cost model; the bottom
  row reuses the same indices against a +60-shifted table view (no
  second index stream at all).
- DMA consolidation: one broadcast DMA per (tap, half) for the bilinear
  weights, one wd write per (cls, quarter), merged idxd/idxT round trips.
- BN: per-block (sum, sumsq) accumulated during evacuation; AllReduce
  (8-core) or local (1-core); chunked apply+writeback.
"""
import numpy as np
import concourse.bass as bass
import concourse.mybir as mybir
import concourse.tile as tile
from concourse.masks import make_identity

AF = mybir.ActivationFunctionType
OP = mybir.AluOpType
F32 = mybir.dt.float32
F16 = mybir.dt.float16
I32 = mybir.dt.int32
I16 = mybir.dt.int16
U32 = mybir.dt.uint32

H = W = 56
HW = H * W            # 3136
PW = 60               # padded row stride
NPAIR = 3776          # pair-table size in u32 (gather touches <= 3772)
NT = 9                # taps
NB = 8                # row blocks of 7 rows
BLK = 392             # 7 rows of 56
NQ = 4                # quarters
QP = HW // NQ         # 784 pixels per quarter
HH = HW // 2          # 1568 pixels per half
CH = 98               # pixels per idx-wrap block (16 blocks per half)
GROUPS = [(0, 3), (3, 3), (6, 3)]           # (first tap, n taps) per gather

# DRAM input registry (kernel-owned; test.py mirrors it for the 1-core sim)
INPUT_SPECS = [
    ("x", [128, HW], F16),
    ("wofft", [128, NT * 54], F16),   # w_off pre-transposed: [c, (t, o)]
    ("boff", [54], F32),
    ("wt", [128, NT * 128], F16),     # w pre-transposed: [c, (t, o)]
    ("b", [128, 1], F32),
    ("gam", [128, 1], F32),
    ("bet", [128, 1], F32),
    ("geom", [36, 1], F32),
    ("bidx", [18, HW], I16),
]
OUT_DT = F16


def host_bidx():
    """Per-pixel raster base of the padded-table index: 60*hh + ww."""
    hh, ww = np.meshgrid(np.arange(H), np.arange(W), indexing="ij")
    b = (60 * hh + ww).reshape(-1).astype(np.int16)
    return np.broadcast_to(b, (18, HW)).copy()


def host_geom():
    """Data-independent per-row bias constants (rows: y 0-17, x 18-35).

    y rows: (t//3) - 1 - 0.5  (the -0.5 makes RNE(v-0.5) = floor(v))
    x rows: (t%3) - 1 - 0.5 + 122  (+122 folds the padded-table origin)
    """
    g = np.zeros((36, 1), np.float32)
    for gg in range(2):
        for t in range(NT):
            g[9 * gg + t, 0] = (t // 3) - 1.5
            g[18 + 9 * gg + t, 0] = (t % 3) + 120.5
    return g


def rap(t, part_off, part_step, part_cnt, free_off, free_dims):
    """Strided AP into tile/AP t. part_* in partition-rows, free in elements."""
    a = t if isinstance(t, bass.AP) else t[:]
    row = a.ap[0][0]
    return bass.AP(tensor=a.tensor, offset=a.offset + part_off * row + free_off,
                   ap=[[part_step * row, part_cnt]] + [list(d) for d in free_dims])


def dap(t, offset, dims):
    """Raw AP into a DRAM tile at elementwise offset with explicit dims."""
    a = t if isinstance(t, bass.AP) else t[:]
    return bass.AP(tensor=a.tensor, offset=a.offset + offset,
                   ap=[list(d) for d in dims])


def emit(tc: tile.TileContext, outs, ins, num_cores: int, dbg=None):
    nc = tc.nc
    (out_d,) = outs
    (x_d, wofft_d, boff_d, wt_d, b_d, gam_d, bet_d, geom_d, bidx_d) = ins

    def dbg_dump(key, src):
        if dbg is not None and key in dbg:
            nc.sync.dma_start(out=dbg[key], in_=src)

    norm = 1.0 / float(num_cores * HW)

    with tc.tile_pool(name="consts", bufs=1) as consts, \
         tc.tile_pool(name="dram", bufs=1, space="DRAM") as dram, \
         tc.tile_pool(name="main", bufs=1) as main:

        # ---------- pair table + border zeroing (early; DVE idle) ----------
        xpair = main.tile([128, NPAIR], U32)
        xpair_h = xpair.bitcast(F16)
        nc.vector.memset(xpair[:, 0:2 * PW], 0)
        nc.vector.memset(xpair[:, 58 * PW:NPAIR], 0)
        nc.vector.memset(rap(xpair, 0, 1, 128, 2 * PW, [[PW, 56], [1, 2]]), 0)
        nc.vector.memset(rap(xpair, 0, 1, 128, 2 * PW + 57, [[PW, 56], [1, 3]]), 0)

        # ---------- per-image accumulators ----------
        outsb = main.tile([128, HW], F16)
        bsum = main.tile([128, NB], F32)
        ssq = main.tile([128, NB], F32)

        with tc.tile_pool(name="psconv", bufs=3, space="PSUM") as psconv, \
             tc.tile_pool(name="psmain", bufs=1, space="PSUM") as psmain, \
             tc.tile_pool(name="pipe", bufs=2) as pipe, \
             tc.tile_pool(name="wsmall", bufs=2) as wsmall, \
             tc.tile_pool(name="dpool", bufs=2, space="DRAM") as dpool, \
             tc.tile_pool(name="gats", bufs=2) as gats, \
             tc.tile_pool(name="gpool", bufs=3) as gpool, \
             tc.tile_pool(name="vpool", bufs=2) as vpool, \
             tc.tile_pool(name="evac", bufs=1) as evac:

            # ---------- x load first (SP DMA queue is in-order) ----------
            xld_pool = tc.tile_pool(name="xld", bufs=1)
            xld = xld_pool.__enter__()
            xraw = xld.tile([128, HW], F16)
            nc.sync.dma_start(out=xraw[:], in_=x_d[:])
            # even-column relu in three row chunks (om q0 needs rows 0-9,
            # q1 needs rows 14-31; start the conv as early as possible)
            for r0, r1 in ((0, 10), (10, 32), (32, 56)):
                nc.scalar.activation(
                    rap(xpair_h, 0, 1, 128, 2 * ((r0 + 2) * PW + 2),
                        [[2 * PW, r1 - r0], [2, W]]),
                    rap(xraw, 0, 1, 128, r0 * W, [[W, r1 - r0], [1, W]]),
                    AF.Relu)
            nc.scalar.activation(
                rap(xpair_h, 0, 1, 128, 2 * (2 * PW + 2) - 2 + 1,
                    [[2 * PW, H], [2, W]]),
                rap(xraw, 0, 1, 128, 0, [[W, H], [1, W]]), AF.Relu)
            xld_pool.__exit__(None, None, None)

            # ---------- weights (host pre-transposed; straight loads) ------
            # act-table priming: first load = sigmoid set (covers relu/
            # identity/square); a sqrt-set dummy fires mid-kernel later.
            ceps = consts.tile([1, 1], F32)
            nc.vector.memset(ceps[:], 1e-5)
            tdum = consts.tile([1, 1], F32)
            nc.scalar.activation(tdum[:], ceps[:], AF.Sigmoid)
            lhsT_all = consts.tile([128, NT * 54], F16)
            nc.sync.dma_start(out=lhsT_all[:], in_=wofft_d[:])
            lhsT_off = [lhsT_all[:, 54 * t:54 * (t + 1)] for t in range(NT)]
            wT = consts.tile([128, NT * 128], F16)
            nc.sync.dma_start(out=wT[:], in_=wt_d[:])

            # ---------- per-partition bias constants ----------
            # om rows: y = 9g+t at 0-17, x at 18-35, mask at 36-53
            # c_int: RNE(pom + c_int) = floor(ys) (x rows also carry +122)
            # c_frac = c_int + 0.5: pom + c_frac = ys (x rows +122)
            bofr = boff_d[:]
            boff_sb = consts.tile([36, 1], F32)
            nc.sync.dma_start(
                out=boff_sb[:],
                in_=bass.AP(tensor=bofr.tensor, offset=bofr.offset,
                            ap=[[1, 36], [0, 1]]))
            cm18 = consts.tile([18, 1], F32)   # mask sigmoid bias (base 0)
            nc.sync.dma_start(
                out=cm18[:],
                in_=bass.AP(tensor=bofr.tensor, offset=bofr.offset + 36,
                            ap=[[1, 18], [0, 1]]))
            geom_sb = consts.tile([36, 1], F32)
            nc.sync.dma_start(out=geom_sb[:], in_=geom_d[:])
            bqall = consts.tile([18, HW], I16)
            nc.sync.dma_start(out=bqall[:], in_=bidx_d[:])
            c_int = consts.tile([36, 1], F32)
            nc.vector.tensor_tensor(out=c_int[:], in0=boff_sb[:],
                                    in1=geom_sb[:], op=OP.add)
            c_frac = consts.tile([36, 1], F32)
            nc.vector.tensor_scalar(out=c_frac[:], in0=c_int[:],
                                    scalar1=0.5, scalar2=None, op0=OP.add)
            b_t = consts.tile([128, 1], F32)
            nc.sync.dma_start(out=b_t[:], in_=b_d[:])
            gam_t = consts.tile([128, 1], F32)
            nc.sync.dma_start(out=gam_t[:], in_=gam_d[:])
            bet_t = consts.tile([128, 1], F32)
            nc.sync.dma_start(out=bet_t[:], in_=bet_d[:])

            def om2_q(q):
                poms = []
                for bl in range(2):
                    blk = 2 * q + bl
                    pom = psconv.tile([54, BLK], F32, tag="pom", name="pom")
                    for t in range(NT):
                        ki, kj = t // 3, t % 3
                        rhs = rap(xpair_h, 0, 1, 128,
                                  2 * ((7 * blk + ki + 1) * PW + kj + 1),
                                  [[2 * PW, 7], [2, W]])
                        nc.tensor.matmul(pom[:], lhsT_off[t], rhs,
                                         start=(t == 0), stop=(t == NT - 1))
                    poms.append(pom)
                return poms

            def pipe_idx_a(q, poms):
                """om PSUM evac + int coordinates for quarter q (Act)."""
                omc = pipe.tile([54, QP], F16, tag="omc", name="omc")
                for bl in range(2):
                    nc.scalar.activation(omc[:, BLK * bl:BLK * (bl + 1)],
                                         poms[bl][:], AF.Identity)
                # yxi = RNE(omc + c_int) = (y0 | x0rel+122), int16
                yxi = pipe.tile([36, QP], I16, tag="sB", name="yxi")
                nc.scalar.activation(yxi[:], omc[0:36, :], AF.Identity,
                                     bias=c_int[:])
                return yxi, omc

            def pipe_idx_b1(q, yxi, eng):
                """x-row align + idx arithmetic for quarter q."""
                xi18 = pipe.tile([18, QP], I16, tag="sB2", name="xi18")
                eng.dma_start(out=xi18[:], in_=yxi[18:36, :])
                # idx_rel = 60*y0rel + (x0rel+122), then + (60*hh+ww)
                idxr = pipe.tile([18, QP], I16, tag="sE2", name="idxr")
                nc.vector.scalar_tensor_tensor(
                    out=idxr[:], in0=yxi[0:18, :], scalar=60, in1=xi18[:],
                    op0=OP.mult, op1=OP.add)
                nc.vector.tensor_tensor(out=idxr[:], in0=idxr[:],
                                        in1=bqall[:, q * QP:(q + 1) * QP],
                                        op=OP.add)
                if q == 0:
                    dbg_dump("idxr0", idxr[:])
                return idxr

            def pipe_idx_b2(q, idxr, idxd_h, eng):
                """idxd scatter for quarter q (one DMA).

                idxd layout: [16pb][18r][98j] with r = 9g+t; quarter q
                fills pb 8*qq .. 8*qq+7."""
                qq = q % 2
                eng.dma_start(
                    out=dap(idxd_h, 8 * qq * 18 * CH,
                            [[CH, 18], [18 * CH, 8], [1, CH]]),
                    in_=idxr[:],
                )

            def pipe_w(q, omc, yxi, wds):
                """Laggable path: bilinear weight products for quarter q."""
                qq = q % 2
                yxf = pipe.tile([36, QP], F32, tag="sC", name="yxf")
                nc.vector.tensor_copy(out=yxf[:], in_=yxi[:])
                # mask: repartition rows 36-53 onto 0-17 (compute partition
                # bases must be 32-aligned), then sigmoid at base 0.
                mraw = wsmall.tile([18, QP], F16, tag="tF", name="mraw",
                                   bufs=1)
                nc.sync.dma_start(out=mraw[:], in_=omc[36:54, :])
                mskh = wsmall.tile([18, QP], F16, tag="tA", name="mskh")
                nc.scalar.activation(mskh[:], mraw[:], AF.Sigmoid,
                                     bias=cm18[:])
                # frac: wfh = (omc + c_frac) - yxf = (wy | wx), fused
                wfh = pipe.tile([36, QP], F16, tag="sD", name="wfh")
                nc.vector.scalar_tensor_tensor(
                    out=wfh[:], in0=omc[0:36, :], scalar=c_frac[:],
                    in1=yxf[:], op0=OP.add, op1=OP.subtract)
                wxh = wsmall.tile([18, QP], F16, tag="tG", name="wxh")
                nc.sync.dma_start(out=wxh[:], in_=wfh[18:36, :])
                bx0 = wsmall.tile([18, QP], F16, tag="tB", name="bx0")
                nc.scalar.activation(bx0[:], wxh[:], AF.Identity,
                                     bias=1.0, scale=-1.0)
                pA = wsmall.tile([18, QP], F16, tag="tD", name="pA")
                nc.vector.tensor_tensor(out=pA[:], in0=wfh[0:18, :],
                                        in1=mskh[:], op=OP.mult)
                ay0 = wsmall.tile([18, QP], F16, tag="tE", name="ay0")
                nc.vector.tensor_tensor(out=ay0[:], in0=mskh[:], in1=pA[:],
                                        op=OP.subtract)
                # wt_cls [18, 2QP] pair-interleaved (w*bx0 even, w*wxh odd)
                wtags = ["tA2", "tB2"]
                for cls, a in enumerate((ay0, pA)):
                    wt = wsmall.tile([18, 2 * QP], F16, tag=wtags[cls],
                                     name=f"wint{cls}")
                    nc.vector.tensor_tensor(
                        out=rap(wt, 0, 1, 18, 0, [[2, QP]]),
                        in0=a[:], in1=bx0[:], op=OP.mult)
                    nc.vector.tensor_tensor(
                        out=rap(wt, 0, 1, 18, 1, [[2, QP]]),
                        in0=a[:], in1=wxh[:], op=OP.mult)
                    # wd layout: [g][t][cls][qq][1568]; rows (9g+t) stride
                    # 6272 is linear across the whole 18-row dim.
                    nc.sync.dma_start(
                        out=dap(wds, cls * 2 * 2 * QP + qq * 2 * QP,
                                [[2 * QP * 2 * 2, 18], [1, 2 * QP]]),
                        in_=wt[:],
                    )
                    if q == 0:
                        dbg_dump(f"wt0c{cls}", wt[:])

            def gath(h, t0, nt, idxT_h, top):
                gt = gpool.tile([128, 3 * HH], U32, tag="gt", name="gt")
                nc.gpsimd.ap_gather(
                    gt[:, :nt * HH],
                    rap(xpair, 0, 1, 128, 0 if top else PW, [[1, 3712]]),
                    idxT_h[:, t0 * CH:(t0 + nt) * CH],
                    channels=128, num_elems=3712, d=1, num_idxs=nt * HH)
                if h == 0 and t0 == 0:
                    dbg_dump("gt0" + ("t" if top else "b"), gt[:])
                return gt

            def wb_tap(h, t, cls, wds):
                """Broadcast one (tap, cls) weight set: [128, 2par x HH].

                Issued on the Act DGE queue so broadcasts prefetch during the
                gathers instead of queueing behind pipeline DMAs on SP."""
                wb = gats.tile([128, 2 * HH], F16, tag="wb", name="wb",
                               bufs=6)
                nc.scalar.dma_start(
                    out=wb[:],
                    in_=dap(wds, t * 2 * QP * 2 * 2 + cls * 2 * QP * 2,
                            [[NT * 2 * QP * 4, 2], [0, 64], [1, 2 * QP * 2]]),
                )
                if h == 0 and t == 0:
                    dbg_dump(f"wb0c{cls}", wb[:])
                return wb

            def tap_compute(trel, gts, wbs, ps_out, first, last):
                """vt products + PE accumulation for one tap."""
                for cls in range(2):
                    gth = gts[cls].bitcast(F16)
                    vt = vpool.tile([128, 2 * HH], F16, tag="vt",
                                    name=f"vt{cls}")
                    nc.vector.tensor_tensor(
                        out=vt[:],
                        in0=rap(gth, 0, 1, 128, 2 * HH * trel,
                                [[2, 16], [32, CH], [1, 2]]),
                        in1=wbs[cls][:],
                        op=OP.mult)
                    for bl in range(4):
                        for par in range(2):
                            rhs = rap(vt, 0, 1, 128, 2 * BLK * bl + par,
                                      [[2, BLK]])
                            nc.tensor.matmul(
                                ps_out[bl][:],
                                wT[:, 128 * tap_compute.t:128 * (tap_compute.t + 1)],
                                rhs,
                                start=(first and cls == 0 and par == 0),
                                stop=(last and cls == 1 and par == 1),
                                skip_group_check=True)

            def evac_h(h, ps_out):
                # bias-add+bsum on Act; square+ssq on DVE (parallel chains)
                for bl in range(4):
                    blk = 4 * h + bl
                    osl = outsb[:, BLK * blk:BLK * (blk + 1)]
                    nc.scalar.activation(osl, ps_out[bl][:], AF.Identity,
                                         bias=b_t[:],
                                         accum_out=bsum[:, blk:blk + 1])
                    sqd = evac.tile([128, BLK], F16, tag="sqd", name="sqd")
                    nc.vector.scalar_tensor_tensor(
                        out=sqd[:], in0=osl, scalar=1.0, in1=osl,
                        op0=OP.mult, op1=OP.mult,
                        accum_out=ssq[:, blk:blk + 1])

            # ---------- staged emission ----------
            # idxd DRAM: per half, [2g][16pb][9t][98] i16
            idxd = [dpool.tile([16, 18 * CH], I16, tag="idxd",
                               name=f"idxd{h}") for h in range(2)]
            # wd DRAM: per half, [2g][9t][2cls][2qq][1568] f16 (18 x 6272)
            wds = [dpool.tile([18, 2 * 2 * QP * 2], F16, tag="wd",
                              name=f"wd{h}") for h in range(2)]
            idxTs = [gats.tile([128, NT * CH], I16, tag="idxT",
                               name=f"idxT{i}") for i in range(2)]

            def idxT_build(h, eng):
                # [128, 9*98]: partition 16k+w holds block w of group k//4
                for g in range(2):
                    eng.dma_start(
                        out=rap(idxTs[h], 64 * g, 1, 64, 0, [[1, NT * CH]]),
                        in_=dap(idxd[h], g * NT * CH,
                                [[0, 4], [18 * CH, 16], [1, NT * CH]]),
                    )

            poms0 = om2_q(0)
            yxi0, omc0 = pipe_idx_a(0, poms0)
            poms1 = om2_q(1)
            yxi1, omc1 = pipe_idx_a(1, poms1)
            idxr0 = pipe_idx_b1(0, yxi0, nc.sync)
            idxr1 = pipe_idx_b1(1, yxi1, nc.sync)
            pipe_idx_b2(0, idxr0, idxd[0], nc.sync)
            pipe_idx_b2(1, idxr1, idxd[0], nc.sync)
            idxT_build(0, nc.sync)
            pipe_w(0, omc0, yxi0, wds[0])
            pipe_w(1, omc1, yxi1, wds[0])

            ps = [None, None]
            ps[0] = [psmain.tile([128, BLK], F32, tag=f"po{bl}",
                                 name=f"po{bl}") for bl in range(4)]

            def half(h, interleave):
                """Gather/compute for one half; interleave() emits the other
                half's producer stages between gather groups."""
                for G, (t0, ntap) in enumerate(GROUPS):
                    gt_top = gath(h, t0, ntap, idxTs[h], True)
                    gt_bot = gath(h, t0, ntap, idxTs[h], False)
                    for trel in range(ntap):
                        t = t0 + trel
                        tap_compute.t = t
                        wbs = (wb_tap(h, t, 0, wds[h]),
                               wb_tap(h, t, 1, wds[h]))
                        tap_compute(trel, (gt_top, gt_bot), wbs, ps[h],
                                    first=(t == 0), last=(t == NT - 1))
                    if interleave is not None:
                        interleave(G)
                evac_h(h, ps[h])

            def mid_stages(G):
                if G == 0:
                    poms2 = om2_q(2)
                    yxi2, omc2 = pipe_idx_a(2, poms2)
                    pipe_idx_b2(2, pipe_idx_b1(2, yxi2, nc.sync),
                                idxd[1], nc.sync)
                    pipe_w(2, omc2, yxi2, wds[1])
                elif G == 1:
                    poms3 = om2_q(3)
                    yxi3, omc3 = pipe_idx_a(3, poms3)
                    pipe_idx_b2(3, pipe_idx_b1(3, yxi3, nc.sync),
                                idxd[1], nc.sync)
                    pipe_w(3, omc3, yxi3, wds[1])
                    nc.scalar.activation(tdum[:], ceps[:], AF.Sqrt)
                    idxT_build(1, nc.sync)
                    ps[1] = [psmain.tile([128, BLK], F32, tag=f"po{bl}",
                                         name=f"po{bl}") for bl in range(4)]

            half(0, mid_stages)
            half(1, None)

        # ---------- stats ----------
        if dbg is not None:
            dbg_dump("outsb", outsb[:])
            dbg_dump("xpair", xpair[:])
        stats = main.tile([128, 2], F32)
        dump8a = main.tile([128, NB], F32)
        dump8b = main.tile([128, NB], F32)
        nc.scalar.activation(dump8a[:], bsum[:], AF.Identity,
                             accum_out=stats[:, 0:1])
        nc.scalar.activation(dump8b[:], ssq[:], AF.Identity,
                             accum_out=stats[:, 1:2])

        if num_cores > 1:
            statd = dram.tile([128, 2], F32)
            statr = dram.tile([128, 2], F32)
            nc.sync.dma_start(out=statd[:], in_=stats[:])
            nc.gpsimd.collective_compute(
                "AllReduce", OP.add,
                replica_groups=[list(range(num_cores))],
                ins=[statd.opt()], outs=[statr.opt()])
            st = main.tile([128, 2], F32)
            nc.sync.dma_start(out=st[:], in_=statr[:])
        else:
            st = stats

        # ---------- finalize BN ----------
        stn = main.tile([128, 2], F32)
        nc.vector.tensor_scalar(out=stn[:], in0=st[:], scalar1=norm,
                                scalar2=None, op0=OP.mult)
        mu2 = main.tile([128, 1], F32)
        nc.vector.tensor_tensor(out=mu2[:], in0=stn[:, 0:1], in1=stn[:, 0:1],
                                op=OP.mult)
        var = main.tile([128, 1], F32)
        nc.vector.tensor_tensor(out=var[:], in0=stn[:, 1:2], in1=mu2[:],
                                op=OP.subtract)
        ceps128 = main.tile([128, 1], F32)
        nc.vector.memset(ceps128[:], 1e-5)
        sd = main.tile([128, 1], F32)
        nc.scalar.activation(sd[:], var[:], AF.Sqrt, bias=ceps128[:])
        rs = main.tile([128, 1], F32)
        nc.vector.reciprocal_approx_fast(rs[:], sd[:])
        scl = main.tile([128, 1], F32)
        nc.vector.tensor_tensor(out=scl[:], in0=rs[:], in1=gam_t[:], op=OP.mult)
        m1 = main.tile([128, 1], F32)
        nc.vector.tensor_tensor(out=m1[:], in0=stn[:, 0:1], in1=scl[:],
                                op=OP.mult)
        bnb = main.tile([128, 1], F32)
        nc.vector.tensor_tensor(out=bnb[:], in0=bet_t[:], in1=m1[:],
                                op=OP.subtract)
        with tc.tile_pool(name="apl", bufs=2) as apl:
            for c in range(2):
                osl = outsb[:, HH * c:HH * (c + 1)]
                oap = apl.tile([128, HH], F16, tag="oap", name="oap")
                if c % 2 == 0:
                    nc.scalar.activation(oap[:], osl, AF.Identity,
                                         bias=bnb[:], scale=scl[:])
                else:
                    nc.vector.tensor_scalar(out=oap[:], in0=osl,
                                            scalar1=scl[:], scalar2=bnb[:],
                                            op0=OP.mult, op1=OP.add)
                eng = nc.sync if c % 2 == 0 else nc.scalar
                eng.dma_start(
                    out=rap(out_d, 0, 1, 128, HH * c, [[1, HH]]),
                    in_=oap[:])


# ----------------------------------------------------------------------------
# Host-side runner: shard batch over 8 cores, compile once, execute via SPMD.
# ----------------------------------------------------------------------------
import concourse.bacc as bacc
from concourse.bass_utils import run_bass_kernel_spmd

B = 8
N_CORES = 8

_CACHE = {}


def _build(num_cores=N_CORES):
    key = ("nc", num_cores)
    if key in _CACHE:
        return _CACHE[key]
    nc = bacc.Bacc("TRN2", target_bir_lowering=False, debug=False,
                   enable_asserts=False, num_devices=num_cores)
    ins = [nc.dram_tensor(n, sh, dt, kind="ExternalInput").ap()
           for n, sh, dt in INPUT_SPECS]
    out_d = nc.dram_tensor("out", [128, HW], OUT_DT, kind="ExternalOutput").ap()
    with tile.TileContext(nc) as tc:
        emit(tc, [out_d], ins, num_cores)
    nc.compile()
    _CACHE[key] = nc
    return nc


def _in_maps(inputs, num_cores=N_CORES):
    x = np.asarray(inputs["x"], np.float32)
    w_off = np.asarray(inputs["w_off"], np.float32)   # [54, 128, 3, 3]
    wofft = np.transpose(w_off.reshape(54, 128, 9), (1, 2, 0))  # [c, t, o]
    w_main = np.asarray(inputs["w"], np.float32)      # [128, 128, 3, 3]
    wtr = np.transpose(w_main.reshape(128, 128, 9), (1, 2, 0))  # [c, t, o]
    base = {
        "wofft": np.ascontiguousarray(wofft.reshape(128, NT * 54))
                 .astype(np.float16),
        "boff": np.ascontiguousarray(np.asarray(inputs["b_off"], np.float32)
                                     .reshape(54)),
        "wt": np.ascontiguousarray(wtr.reshape(128, NT * 128))
              .astype(np.float16),
        "b": np.ascontiguousarray(np.asarray(inputs["b"], np.float32)
                                  .reshape(128, 1)),
        "gam": np.ascontiguousarray(np.asarray(inputs["gamma"], np.float32)
                                    .reshape(128, 1)),
        "bet": np.ascontiguousarray(np.asarray(inputs["beta"], np.float32)
                                    .reshape(128, 1)),
        "geom": host_geom(),
        "bidx": host_bidx(),
    }
    return [dict(base, x=np.ascontiguousarray(x[c].reshape(128, HW))
                 .astype(np.float16))
            for c in range(num_cores)]


def run(inputs, trace=False, **kw):
    """Run the SPMD kernel; returns (output [8,128,56,56], BassKernelResults)."""
    nc = _build(N_CORES)
    res = run_bass_kernel_spmd(nc, _in_maps(inputs), list(range(N_CORES)),
                               trace=trace, **kw)
    out = np.stack([res.results[c]["out"].reshape(128, H, W)
                    for c in range(N_CORES)]).astype(np.float32)
    return out, res


def kernel(**inputs) -> np.ndarray:
    out, _ = run(inputs, trace=False)
    return out
